# revision 14
# baseline (speedup 1.0000x reference)
"""Distributed Trainium2 Bass kernel for nn_Attention_74732430950409.

Single-query MHA with RoPE'd keys/values. The four projection weights are
folded algebraically onto the tiny query side:

  qtil[:,h] = Wk^T (Wk_mha[h]^T qh[h]),  qh = Wq_mha (Wq x)
  logits[s,h] = rope(keys)[s,:] . qtil[:,h] / sqrt(128)
  w = exp(logits)          (no max subtraction; |logits| < ~7)
  u[h,:] = sum_s w[s,h] * rope(states)[s,:] ; l[h] = sum_s w[s,h]
  z[h,:] = (u[h,:] @ Wv.T) / l[h]
  attn[h,:] = z[h,:] @ Wv_mha[h].T
  out = attn.flat @ Wo.T + x

Sharding: sequence-sharded (1024 rows/core) for keys/states; q-path uses
  AG(q shard)   -> every core has q (exact)
  local: qh, tmp for this core's 2 heads (head-sharded, exact)
  A2A(tmp)      -> every core gets tmp[all 16 heads, its 256 j-cols]
  AR(qtilT)     -> partial Wk^T contraction summed
value path:
  AR(uT|l) f32  -> z/attn partial per n-shard -> AR(attnT) -> out shard.
A tiny dummy AllGather fires first to absorb the CC entry barrier / skew.
Compute dtype bf16 (f32 PSUM accum).
"""

import sys
import numpy as np

for p in ("/opt/trn_rl_repo",):
    if p not in sys.path:
        sys.path.insert(0, p)

import ml_dtypes

BF16 = ml_dtypes.bfloat16

NUM_HEADS = 16
QK = 2048
VO = 2048
S = 8192
NC = 8
S_LOC = S // NC          # 1024
SH = VO // NC            # 256
DQ = QK // NUM_HEADS     # 128
HALF = VO // 2           # 1024
ROPE_THETA = 10000.0

_cache = {}


def _build():
    import concourse.bass as bass
    import concourse.mybir as mybir
    import concourse.bacc as bacc
    import concourse.tile as tile

    f32 = mybir.dt.float32
    bf16 = mybir.dt.bfloat16
    AF = mybir.ActivationFunctionType
    ALU = mybir.AluOpType
    PSUM = bass.MemorySpace.PSUM

    nc = bacc.Bacc(None, target_bir_lowering=False)

    # ---------------- DRAM parameters (per-core shards) ----------------
    xq_d = nc.dram_tensor("xq", [QK], bf16, kind="ExternalInput")
    xo_d = nc.dram_tensor("xo", [SH], f32, kind="ExternalInput")
    identb_d = nc.dram_tensor("identb", [128, 128], bf16, kind="ExternalInput")
    identf_d = nc.dram_tensor("identf", [128, 128], f32, kind="ExternalInput")
    wq_d = nc.dram_tensor("wq", [QK, SH], bf16, kind="ExternalInput")      # Wq[rs].T
    wqm_d = nc.dram_tensor("wqm", [QK, SH], bf16, kind="ExternalInput")    # Wq_mha[rs].T
    wkm_d = nc.dram_tensor("wkm", [SH, QK], bf16, kind="ExternalInput")    # Wk_mha[rs]
    wk_d = nc.dram_tensor("wk", [SH, VO], bf16, kind="ExternalInput")      # Wk[rs]
    keysT_d = nc.dram_tensor("keysT", [QK, S_LOC], bf16, kind="ExternalInput")
    ck_d = nc.dram_tensor("ck", [HALF, S_LOC], bf16, kind="ExternalInput")
    sk_d = nc.dram_tensor("sk", [HALF, S_LOC], bf16, kind="ExternalInput")
    states_d = nc.dram_tensor("states", [S_LOC, VO], bf16, kind="ExternalInput")
    cs_d = nc.dram_tensor("cs", [S_LOC, HALF], bf16, kind="ExternalInput")
    ss_d = nc.dram_tensor("ss", [S_LOC, HALF], bf16, kind="ExternalInput")
    wvT_d = nc.dram_tensor("wvT", [VO, SH], bf16, kind="ExternalInput")    # Wv[rs].T
    wvm_d = nc.dram_tensor("wvm", [SH, VO], bf16, kind="ExternalInput")    # Wv_mha[:,rs].T
    woT_d = nc.dram_tensor("woT", [VO, SH], bf16, kind="ExternalInput")    # Wo[rs].T
    out_d = nc.dram_tensor("out", [1, SH], f32, kind="ExternalOutput")
    DEBUG = _cache.get("debug", False)
    if DEBUG:
        dqt_d = nc.dram_tensor("dbg_qt", [16, 128, NUM_HEADS], f32, kind="ExternalOutput")
        dtm_d = nc.dram_tensor("dbg_tmpa", [NUM_HEADS, SH], f32, kind="ExternalOutput")
        dqg_d = nc.dram_tensor("dbg_qg", [1, QK], f32, kind="ExternalOutput")

    RG = [list(range(NC))]
    SCALE = 1.0 / float(np.sqrt(DQ))

    with tile.TileContext(nc) as tc:
        with (
            tc.tile_pool(name="kbuf", bufs=16) as kbuf,
            tc.tile_pool(name="sbuf_s", bufs=8) as sbuf_s,
            tc.tile_pool(name="tabs", bufs=1) as tabs,
            tc.tile_pool(name="wts", bufs=1) as wts,
            tc.tile_pool(name="wts2", bufs=1) as wts2,
            tc.tile_pool(name="tmps", bufs=2) as tmps,
            tc.tile_pool(name="small", bufs=1) as small,
            tc.tile_pool(name="psA", bufs=4, space=PSUM) as psA,
            tc.tile_pool(name="psB", bufs=3, space=PSUM) as psB,
            tc.tile_pool(name="dram", bufs=1, space="DRAM") as dram,
        ):
            # ---------------- collective bounce buffers ----------------
            bdum_in = dram.tile([1, 16], bf16)
            bdum_out = dram.tile([1, 128], bf16)
            bq_in = dram.tile([1, SH], bf16)
            bq_out = dram.tile([1, QK], bf16)
            ba_in = dram.tile([NC, 2, SH], bf16)      # send: [dest, h2, j_local]
            ba_out = dram.tile([NUM_HEADS, SH], bf16)  # recv: [h, j_local]
            bqt_in = dram.tile([16, 128, NUM_HEADS], bf16)
            bqt_out = dram.tile([16, 128, NUM_HEADS], bf16)
            bu_in = dram.tile([128, 16 * NUM_HEADS + 1], f32)
            bu_out = dram.tile([128, 16 * NUM_HEADS + 1], f32)
            bat_in = dram.tile([DQ, NUM_HEADS], f32)
            bat_out = dram.tile([DQ, NUM_HEADS], f32)

            # ---------------- small persistent SBUF tiles ----------------
            dum_sb = small.tile([1, 16], bf16, tag="dum")
            x_sb = small.tile([128, 16], bf16, tag="x")
            xo_sb = small.tile([1, SH], f32, tag="xo")
            ident_b = small.tile([16, 16], bf16, tag="idb")
            ident_f = small.tile([16, 16], f32, tag="idf")
            q_sb = small.tile([1, SH], bf16, tag="q")
            qg_sb = small.tile([128, 16], bf16, tag="qg")
            qhT_sb = small.tile([128, 2], bf16, tag="qhT")
            tmp_sb = [small.tile([1, QK], bf16, tag=f"tmp{i}", name=f"tmp_sb{i}")
                      for i in range(2)]
            tmpa_sb = small.tile([NUM_HEADS, SH], bf16, tag="tmpa")
            tpT_sb = small.tile([128, 2, NUM_HEADS], bf16, tag="tpT")
            qtp_sb = small.tile([128, 16, NUM_HEADS], bf16, tag="qtp")
            qtilT_sb = small.tile([128, 16, NUM_HEADS], bf16, tag="qtilT")
            w_sb = small.tile([NUM_HEADS, S_LOC], bf16, tag="w")
            l0_sb = small.tile([NUM_HEADS, 1], f32, tag="l0")
            l1_sb = small.tile([NUM_HEADS, 1], f32, tag="l1")
            lp_sb = small.tile([NUM_HEADS, 1], f32, tag="lp")
            wT_sb = small.tile([128, 8, NUM_HEADS], bf16, tag="wT")
            u_sb = small.tile([NUM_HEADS, VO], f32, tag="u")
            uT_sb = small.tile([128, 16, NUM_HEADS], f32, tag="uT")
            uT_bf = small.tile([128, 16, NUM_HEADS], bf16, tag="uTb")
            l_sb = small.tile([NUM_HEADS, 1], f32, tag="l")
            rl_sb = small.tile([NUM_HEADS, 1], f32, tag="rl")
            z_sb = small.tile([NUM_HEADS, SH], bf16, tag="z")
            zT_sb = small.tile([128, 2, NUM_HEADS], bf16, tag="zT")
            atp_sb = small.tile([128, NUM_HEADS], f32, tag="atp")
            atT_bf = small.tile([128, NUM_HEADS], bf16, tag="atTb")
            out_sb = small.tile([1, SH], f32, tag="out")

            # ---------------- dummy collective: absorb CC entry barrier ----
            nc.vector.memset(dum_sb[:], 0)
            nc.scalar.dma_start(bdum_in[:], dum_sb[:])
            nc.gpsimd.collective_compute(
                "AllGather", ALU.bypass, ins=[bdum_in[:].opt()],
                outs=[bdum_out[:].opt()], replica_groups=RG)

            # ---------------- DMA priority stream (sync queue) -------------
            nc.sync.dma_start(x_sb[:], xq_d[:].rearrange("(f p) -> p f", p=128))
            nc.sync.dma_start(ident_b[:], identb_d[0:16, 0:16])
            nc.sync.dma_start(ident_f[:], identf_d[0:16, 0:16])
            nc.sync.dma_start(xo_sb[:], xo_d[:].rearrange("(a n) -> a n", a=1))

            wq_sb = wts.tile([128, 16, SH], bf16, tag="wq")
            wqm_sb = wts.tile([128, 16, SH], bf16, tag="wqm")
            wkm_sb = wts.tile([128, 2, QK], bf16, tag="wkm")
            wk_sb = wts.tile([128, 2, VO], bf16, tag="wk")
            nc.sync.dma_start(wq_sb[:], wq_d[:, :].rearrange("(kc p) n -> p kc n", p=128))
            nc.sync.dma_start(wqm_sb[:], wqm_d[:, :].rearrange("(kc p) m -> p kc m", p=128))
            nc.sync.dma_start(wkm_sb[:], wkm_d[:, :].rearrange("(h2 m) j -> m h2 j", m=128))
            nc.sync.dma_start(wk_sb[:], wk_d[:, :].rearrange("(jc p) n -> p jc n", p=128))

            # keys + rope tables, interleaved so rope can start early
            ck_sb = tabs.tile([128, 8, S_LOC], bf16, tag="ck")
            sk_sb = tabs.tile([128, 8, S_LOC], bf16, tag="sk")
            kt = [kbuf.tile([128, S_LOC], bf16, tag="kt", name=f"kt{i}")
                  for i in range(16)]
            for t in range(8):
                nc.sync.dma_start(
                    ck_sb[:, t, :],
                    ck_d[t * 128:(t + 1) * 128, :])
                nc.sync.dma_start(
                    sk_sb[:, t, :],
                    sk_d[t * 128:(t + 1) * 128, :])
                nc.sync.dma_start(kt[t][:], keysT_d[t * 128:(t + 1) * 128, :])
                nc.sync.dma_start(
                    kt[t + 8][:], keysT_d[(t + 8) * 128:(t + 9) * 128, :])

            # states + tables (cs/ss reuse the ck/sk slots after keys rope)
            cs_sb = tabs.tile([128, 8, HALF], bf16, tag="ck")
            ss_sb = tabs.tile([128, 8, HALF], bf16, tag="sk")
            st = [sbuf_s.tile([128, VO], bf16, tag="st", name=f"st{i}")
                  for i in range(8)]
            for t in range(8):
                nc.sync.dma_start(cs_sb[:, t, :], cs_d[t * 128:(t + 1) * 128, :])
                nc.sync.dma_start(ss_sb[:, t, :], ss_d[t * 128:(t + 1) * 128, :])
                nc.sync.dma_start(st[t][:], states_d[t * 128:(t + 1) * 128, :])

            # epilogue weights (separate pool: no reuse wait)
            wvT_sb = wts2.tile([128, 16, SH], bf16, tag="wvT")
            wvm_sb = wts2.tile([128, 2, VO], bf16, tag="wvm")
            woT_sb = wts2.tile([128, 16, SH], bf16, tag="woT")
            nc.sync.dma_start(wvT_sb[:], wvT_d[:, :].rearrange("(ic p) n -> p ic n", p=128))
            nc.sync.dma_start(wvm_sb[:], wvm_d[:, :].rearrange("(jc p) m -> p jc m", p=128))
            nc.sync.dma_start(woT_sb[:], woT_d[:, :].rearrange("(mc p) n -> p mc n", p=128))

            # ---------------- q shard: q[rs] = x @ Wq[rs].T  -> [1, 256] ----
            q_ps = psB.tile([1, SH], f32, tag="pB")
            for kc in range(16):
                nc.tensor.matmul(q_ps[:], x_sb[:, kc:kc + 1], wq_sb[:, kc, :],
                                 start=(kc == 0), stop=(kc == 15))
            nc.scalar.activation(q_sb[:], q_ps[:], AF.Copy)
            nc.scalar.dma_start(bq_in[:], q_sb[:])
            nc.gpsimd.collective_compute(
                "AllGather", ALU.bypass, ins=[bq_in[:].opt()],
                outs=[bq_out[:].opt()], replica_groups=RG)
            nc.gpsimd.dma_start(
                qg_sb[:], bq_out[:, :].rearrange("o (kc p) -> p (o kc)", p=128))

            # ---------------- qh for my 2 heads: qhT [128, 2] ----------------
            qh_ps = psB.tile([128, 2], f32, tag="pB")
            for mc in range(2):
                for kc in range(16):
                    nc.tensor.matmul(qh_ps[:, mc:mc + 1],
                                     wqm_sb[:, kc, mc * 128:(mc + 1) * 128],
                                     qg_sb[:, kc:kc + 1],
                                     start=(kc == 0), stop=(kc == 15))
            nc.scalar.activation(qhT_sb[:], qh_ps[:], AF.Copy)

            # ---------------- tmp[h2, :] = qh[h2] @ Wk_mha[h2-block] --------
            for h2 in range(2):
                for fc in range(4):
                    tm_ps = psB.tile([1, 512], f32, tag="pB")
                    nc.tensor.matmul(tm_ps[:],
                                     qhT_sb[:, h2:h2 + 1],
                                     wkm_sb[:, h2, fc * 512:(fc + 1) * 512],
                                     start=True, stop=True)
                    nc.scalar.activation(tmp_sb[h2][:, fc * 512:(fc + 1) * 512],
                                         tm_ps[:], AF.Copy)
                nc.scalar.dma_start(ba_in[:, h2, :], tmp_sb[h2][:])
            nc.gpsimd.collective_compute(
                "AllToAll", ALU.bypass, ins=[ba_in[:].opt()],
                outs=[ba_out[:].opt()], replica_groups=RG)
            nc.gpsimd.dma_start(tmpa_sb[:], ba_out[:, :])

            # tmpa [16, 256] -> tpT [128, 2, 16] via PE transpose
            for jc in range(2):
                tr_ps = psB.tile([128, NUM_HEADS], bf16, tag="pB")
                nc.tensor.transpose(tr_ps[:], tmpa_sb[:, jc * 128:(jc + 1) * 128],
                                    ident_b[:])
                nc.scalar.activation(tpT_sb[:, jc, :], tr_ps[:], AF.Copy)

            # ---------------- qtil partial = Wk[rs]^T-contract --------------
            for ic in range(16):
                qt_ps = psB.tile([128, NUM_HEADS], f32, tag="pB")
                for jc in range(2):
                    nc.tensor.matmul(qt_ps[:], wk_sb[:, jc, ic * 128:(ic + 1) * 128],
                                     tpT_sb[:, jc, :], start=(jc == 0), stop=(jc == 1))
                nc.scalar.activation(qtp_sb[:, ic, :], qt_ps[:], AF.Copy)
            nc.scalar.dma_start(
                bqt_in[:, :, :].rearrange("ic p h -> p ic h"), qtp_sb[:])
            nc.gpsimd.collective_compute(
                "AllReduce", ALU.add, ins=[bqt_in[:].opt()],
                outs=[bqt_out[:].opt()], replica_groups=RG)
            nc.gpsimd.dma_start(
                qtilT_sb[:], bqt_out[:, :, :].rearrange("ic p h -> p ic h"))

            # ---------------- rope keys in place (pairs t, t+8) -------------
            for t in range(8):
                a, b = kt[t], kt[t + 8]
                t1 = tmps.tile([128, S_LOC], bf16, tag="rt")
                t2 = tmps.tile([128, S_LOC], bf16, tag="rt")
                nc.vector.tensor_mul(t1[:], a[:], sk_sb[:, t, :])
                nc.vector.tensor_mul(a[:], a[:], ck_sb[:, t, :])
                nc.vector.tensor_mul(t2[:], b[:], sk_sb[:, t, :])
                nc.vector.tensor_sub(a[:], a[:], t2[:])
                nc.vector.tensor_mul(b[:], b[:], ck_sb[:, t, :])
                nc.vector.tensor_add(b[:], b[:], t1[:])

            # ---------------- logits + exp ----------------
            for sc in range(2):
                lg_ps = psA.tile([NUM_HEADS, 512], f32, tag="pA")
                for ic in range(16):
                    nc.tensor.matmul(lg_ps[:], qtilT_sb[:, ic, :],
                                     kt[ic][:, sc * 512:(sc + 1) * 512],
                                     start=(ic == 0), stop=(ic == 15))
                nc.scalar.activation(w_sb[:, sc * 512:(sc + 1) * 512], lg_ps[:],
                                     AF.Exp, scale=SCALE,
                                     accum_out=(l0_sb[:] if sc == 0 else l1_sb[:]))
            nc.vector.tensor_add(lp_sb[:], l0_sb[:], l1_sb[:])

            # wT via PE transpose: [16,128] slices -> [128,16]
            for sb in range(8):
                tr_ps = psB.tile([128, NUM_HEADS], bf16, tag="pB")
                nc.tensor.transpose(tr_ps[:], w_sb[:, sb * 128:(sb + 1) * 128],
                                    ident_b[:])
                nc.scalar.activation(wT_sb[:, sb, :], tr_ps[:], AF.Copy)

            # ---------------- rope states in place -------------
            for t in range(8):
                s_ = st[t]
                a = s_[:, 0:HALF]
                b = s_[:, HALF:VO]
                t1 = tmps.tile([128, HALF], bf16, tag="rt")
                t2 = tmps.tile([128, HALF], bf16, tag="rt")
                nc.vector.tensor_mul(t1[:], a, ss_sb[:, t, :])
                nc.vector.tensor_mul(a, a, cs_sb[:, t, :])
                nc.vector.tensor_mul(t2[:], b, ss_sb[:, t, :])
                nc.vector.tensor_sub(a, a, t2[:])
                nc.vector.tensor_mul(b, b, cs_sb[:, t, :])
                nc.vector.tensor_add(b, b, t1[:])

            # ---------------- u = wT.T @ states_pe  [16, 2048] f32 ----------
            u_ps = [psA.tile([NUM_HEADS, 512], f32, tag="pA", name=f"u_ps{i}")
                    for i in range(4)]
            for sb in range(8):
                for nch in range(4):
                    nc.tensor.matmul(u_ps[nch][:], wT_sb[:, sb, :],
                                     st[sb][:, nch * 512:(nch + 1) * 512],
                                     start=(sb == 0), stop=(sb == 7))
            for nch in range(4):
                nc.scalar.activation(u_sb[:, nch * 512:(nch + 1) * 512],
                                     u_ps[nch][:], AF.Copy)

            # uT via PE transpose (f32), then AR(uT|l)
            for ic in range(16):
                tr_ps = psB.tile([128, NUM_HEADS], f32, tag="pB")
                nc.tensor.transpose(tr_ps[:], u_sb[:, ic * 128:(ic + 1) * 128],
                                    ident_f[:])
                nc.scalar.activation(uT_sb[:, ic, :], tr_ps[:], AF.Copy)
            nc.scalar.dma_start(
                bu_in[:, 0:256].rearrange("p (ic h) -> p ic h", ic=16), uT_sb[:])
            nc.scalar.dma_start(bu_in[0:NUM_HEADS, 256:257], lp_sb[:])
            nc.gpsimd.collective_compute(
                "AllReduce", ALU.add, ins=[bu_in[:].opt()],
                outs=[bu_out[:].opt()], replica_groups=RG)
            nc.gpsimd.dma_start(
                uT_bf[:], bu_out[:, 0:256].rearrange("p (ic h) -> p ic h", ic=16))
            nc.gpsimd.dma_start(l_sb[:], bu_out[0:NUM_HEADS, 256:257])
            nc.vector.reciprocal(rl_sb[:], l_sb[:])

            # ---------------- z = (u @ Wv[rs].T) / l  [16, 256] -------------
            z_ps = psB.tile([NUM_HEADS, SH], f32, tag="pB")
            for ic in range(16):
                nc.tensor.matmul(z_ps[:], uT_bf[:, ic, :], wvT_sb[:, ic, :],
                                 start=(ic == 0), stop=(ic == 15))
            nc.scalar.activation(z_sb[:], z_ps[:], AF.Copy, scale=rl_sb[:])

            # zT
            for jc in range(2):
                tr_ps = psB.tile([128, NUM_HEADS], bf16, tag="pB")
                nc.tensor.transpose(tr_ps[:], z_sb[:, jc * 128:(jc + 1) * 128],
                                    ident_b[:])
                nc.scalar.activation(zT_sb[:, jc, :], tr_ps[:], AF.Copy)

            # ---------------- attn partial [128 d, 16 h] --------------------
            at_ps = psB.tile([128, NUM_HEADS], f32, tag="pB")
            for h in range(NUM_HEADS):
                for jc in range(2):
                    nc.tensor.matmul(at_ps[:, h:h + 1],
                                     wvm_sb[:, jc, h * 128:(h + 1) * 128],
                                     zT_sb[:, jc, h:h + 1],
                                     start=(jc == 0), stop=(jc == 1))
            nc.scalar.activation(atp_sb[:], at_ps[:], AF.Copy)
            nc.scalar.dma_start(bat_in[:], atp_sb[:])
            nc.gpsimd.collective_compute(
                "AllReduce", ALU.add, ins=[bat_in[:].opt()],
                outs=[bat_out[:].opt()], replica_groups=RG)
            nc.gpsimd.dma_start(atT_bf[:], bat_out[:, :])

            if DEBUG:
                nc.sync.dma_start(dqg_d[:, :], bq_out[:, :])
                nc.sync.dma_start(dtm_d[:, :], ba_out[:, :])
                nc.sync.dma_start(dqt_d[:, :, :], bqt_out[:, :, :])

            # ---------------- out = attn @ Wo[rs].T + x[rs] ------------------
            o_ps = psB.tile([1, SH], f32, tag="pB")
            for h in range(NUM_HEADS):
                nc.tensor.matmul(o_ps[:], atT_bf[:, h:h + 1], woT_sb[:, h, :],
                                 start=(h == 0), stop=(h == NUM_HEADS - 1))
            nc.vector.tensor_add(out_sb[:], o_ps[:], xo_sb[:])
            nc.sync.dma_start(out_d[:, :], out_sb[:])

    nc.compile()
    return nc


def _tables():
    half = HALF
    freqs = 1.0 / (ROPE_THETA ** (np.arange(half, dtype=np.float32) * 2.0 / VO))
    ang = np.outer(np.arange(S, dtype=np.float32), freqs).astype(np.float32)
    return np.cos(ang), np.sin(ang)


def kernel(x, keys, states, Wq, Wk, Wv, Wq_mha, Wk_mha, Wv_mha, Wo):
    from concourse import bass_utils

    if "nc" not in _cache:
        _cache["nc"] = _build()
    nc = _cache["nc"]

    x = np.asarray(x, np.float32)
    keys = np.asarray(keys, np.float32)
    states = np.asarray(states, np.float32)
    cos_t, sin_t = _tables()

    ident = np.eye(128, dtype=np.float32)
    in_maps = []
    for c in range(NC):
        rs = slice(c * SH, (c + 1) * SH)
        ss_ = slice(c * S_LOC, (c + 1) * S_LOC)
        cosc = cos_t[ss_]
        sinc = sin_t[ss_]
        m = {
            "xq": x.astype(BF16),
            "xo": np.ascontiguousarray(x[rs]),
            "identb": ident.astype(BF16),
            "identf": ident,
            "wq": np.ascontiguousarray(Wq[rs].T).astype(BF16),
            "wqm": np.ascontiguousarray(Wq_mha[rs].T).astype(BF16),
            "wkm": np.ascontiguousarray(Wk_mha[rs]).astype(BF16),
            "wk": np.ascontiguousarray(Wk[rs]).astype(BF16),
            "keysT": np.ascontiguousarray(keys[ss_].T).astype(BF16),
            "ck": np.ascontiguousarray(cosc.T).astype(BF16),
            "sk": np.ascontiguousarray(sinc.T).astype(BF16),
            "states": np.ascontiguousarray(states[ss_]).astype(BF16),
            "cs": np.ascontiguousarray(cosc).astype(BF16),
            "ss": np.ascontiguousarray(sinc).astype(BF16),
            "wvT": np.ascontiguousarray(Wv[rs].T).astype(BF16),
            "wvm": np.ascontiguousarray(Wv_mha[:, rs].T).astype(BF16),
            "woT": np.ascontiguousarray(Wo[rs].T).astype(BF16),
        }
        in_maps.append(m)

    global _last_in_maps, _last_res
    _last_in_maps = in_maps
    res = bass_utils.run_bass_kernel_spmd(nc, in_maps, core_ids=list(range(NC)))
    _last_res = res
    out = np.concatenate([np.asarray(res.results[c]["out"]).reshape(-1) for c in range(NC)])
    return out[None, :].astype(np.float32)


# revision 15
# speedup vs baseline: 1.0720x; 1.0720x over previous
"""Distributed Trainium2 Bass kernel for nn_Attention_74732430950409.

Single-query MHA with RoPE'd keys/values. The four projection weights are
folded algebraically onto the tiny query side:

  qtil[:,h] = Wk^T (Wk_mha[h]^T qh[h]),  qh = Wq_mha (Wq x)
  logits[s,h] = rope(keys)[s,:] . qtil[:,h] / sqrt(128)
  w = exp(logits)          (no max subtraction; |logits| < ~7)
  u[h,:] = sum_s w[s,h] * rope(states)[s,:] ; l[h] = sum_s w[s,h]
  z[h,:] = (u[h,:] @ Wv.T) / l[h]
  attn[h,:] = z[h,:] @ Wv_mha[h].T
  out = attn.flat @ Wo.T + x

Sharding: sequence-sharded (1024 rows/core) for keys/states; q-path uses
  AG(q shard)   -> every core has q (exact)
  local: qh, tmp for this core's 2 heads (head-sharded, exact)
  A2A(tmp)      -> every core gets tmp[all 16 heads, its 256 j-cols]
  AR(qtilT)     -> partial Wk^T contraction summed
value path:
  AR(uT|l) f32  -> z/attn partial per n-shard -> AR(attnT) -> out shard.
A tiny dummy AllGather fires first to absorb the CC entry barrier / skew.
Compute dtype bf16 (f32 PSUM accum).
"""

import sys
import numpy as np

for p in ("/opt/trn_rl_repo",):
    if p not in sys.path:
        sys.path.insert(0, p)

import ml_dtypes

BF16 = ml_dtypes.bfloat16

NUM_HEADS = 16
QK = 2048
VO = 2048
S = 8192
NC = 8
S_LOC = S // NC          # 1024
SH = VO // NC            # 256
DQ = QK // NUM_HEADS     # 128
HALF = VO // 2           # 1024
ROPE_THETA = 10000.0

_cache = {}


def _build():
    import concourse.bass as bass
    import concourse.mybir as mybir
    import concourse.bacc as bacc
    import concourse.tile as tile

    f32 = mybir.dt.float32
    bf16 = mybir.dt.bfloat16
    AF = mybir.ActivationFunctionType
    ALU = mybir.AluOpType
    PSUM = bass.MemorySpace.PSUM

    nc = bacc.Bacc(None, target_bir_lowering=False)

    # ---------------- DRAM parameters (per-core shards) ----------------
    xq_d = nc.dram_tensor("xq", [QK], bf16, kind="ExternalInput")
    xo_d = nc.dram_tensor("xo", [SH], f32, kind="ExternalInput")
    identb_d = nc.dram_tensor("identb", [128, 128], bf16, kind="ExternalInput")
    identf_d = nc.dram_tensor("identf", [128, 128], f32, kind="ExternalInput")
    wq_d = nc.dram_tensor("wq", [QK, SH], bf16, kind="ExternalInput")      # Wq[rs].T
    wqm_d = nc.dram_tensor("wqm", [SH, QK], bf16, kind="ExternalInput")    # Wq_mha[:,rs].T
    wkm_d = nc.dram_tensor("wkm", [QK, SH], bf16, kind="ExternalInput")    # Wk_mha[:,rs]
    wk_d = nc.dram_tensor("wk", [SH, VO], bf16, kind="ExternalInput")      # Wk[rs]
    keysT_d = nc.dram_tensor("keysT", [QK, S_LOC], bf16, kind="ExternalInput")
    ck_d = nc.dram_tensor("ck", [HALF, S_LOC], bf16, kind="ExternalInput")
    sk_d = nc.dram_tensor("sk", [HALF, S_LOC], bf16, kind="ExternalInput")
    states_d = nc.dram_tensor("states", [S_LOC, VO], bf16, kind="ExternalInput")
    cs_d = nc.dram_tensor("cs", [S_LOC, HALF], bf16, kind="ExternalInput")
    ss_d = nc.dram_tensor("ss", [S_LOC, HALF], bf16, kind="ExternalInput")
    wvT_d = nc.dram_tensor("wvT", [VO, SH], bf16, kind="ExternalInput")    # Wv[rs].T
    wvm_d = nc.dram_tensor("wvm", [SH, VO], bf16, kind="ExternalInput")    # Wv_mha[:,rs].T
    woT_d = nc.dram_tensor("woT", [VO, SH], bf16, kind="ExternalInput")    # Wo[rs].T
    out_d = nc.dram_tensor("out", [1, SH], f32, kind="ExternalOutput")
    DEBUG = _cache.get("debug", False)
    if DEBUG:
        dqt_d = nc.dram_tensor("dbg_qt", [16, 128, NUM_HEADS], f32, kind="ExternalOutput")
        dtm_d = nc.dram_tensor("dbg_tmpa", [NUM_HEADS, SH], f32, kind="ExternalOutput")
        dqg_d = nc.dram_tensor("dbg_qg", [1, QK], f32, kind="ExternalOutput")

    RG = [list(range(NC))]
    SCALE = 1.0 / float(np.sqrt(DQ))

    with tile.TileContext(nc) as tc:
        with (
            tc.tile_pool(name="kbuf", bufs=16) as kbuf,
            tc.tile_pool(name="sbuf_s", bufs=8) as sbuf_s,
            tc.tile_pool(name="tabs", bufs=1) as tabs,
            tc.tile_pool(name="wts", bufs=1) as wts,
            tc.tile_pool(name="wts2", bufs=1) as wts2,
            tc.tile_pool(name="tmps", bufs=2) as tmps,
            tc.tile_pool(name="small", bufs=1) as small,
            tc.tile_pool(name="psA", bufs=4, space=PSUM) as psA,
            tc.tile_pool(name="psB", bufs=3, space=PSUM) as psB,
            tc.tile_pool(name="dram", bufs=1, space="DRAM") as dram,
        ):
            # ---------------- collective bounce buffers ----------------
            bdum_in = dram.tile([1, 16], bf16)
            bdum_out = dram.tile([1, 128], bf16)
            bqh_in = dram.tile([128, NUM_HEADS], f32)
            bqh_out = dram.tile([128, NUM_HEADS], f32)
            bqt_in = dram.tile([16, 128, NUM_HEADS], bf16)
            bqt_out = dram.tile([16, 128, NUM_HEADS], bf16)
            bu_in = dram.tile([128, 16 * NUM_HEADS + 1], f32)
            bu_out = dram.tile([128, 16 * NUM_HEADS + 1], f32)
            bat_in = dram.tile([DQ, NUM_HEADS], f32)
            bat_out = dram.tile([DQ, NUM_HEADS], f32)

            # ---------------- small persistent SBUF tiles ----------------
            dum_sb = small.tile([1, 16], bf16, tag="dum")
            x_sb = small.tile([128, 16], bf16, tag="x")
            xo_sb = small.tile([1, SH], f32, tag="xo")
            ident_b = small.tile([16, 16], bf16, tag="idb")
            ident_f = small.tile([16, 16], f32, tag="idf")
            qT_sb = small.tile([128, 2], bf16, tag="qT")
            qhp_sb = small.tile([128, NUM_HEADS], f32, tag="qhp")
            qhT_sb = small.tile([128, NUM_HEADS], bf16, tag="qhT")
            tmpT_sb = small.tile([128, 2, NUM_HEADS], bf16, tag="tmpT")
            qtp_sb = small.tile([128, 16, NUM_HEADS], bf16, tag="qtp")
            qtilT_sb = small.tile([128, 16, NUM_HEADS], bf16, tag="qtilT")
            w_sb = small.tile([NUM_HEADS, S_LOC], bf16, tag="w")
            l0_sb = small.tile([NUM_HEADS, 1], f32, tag="l0")
            l1_sb = small.tile([NUM_HEADS, 1], f32, tag="l1")
            lp_sb = small.tile([NUM_HEADS, 1], f32, tag="lp")
            wT_sb = small.tile([128, 8, NUM_HEADS], bf16, tag="wT")
            u_sb = small.tile([NUM_HEADS, VO], f32, tag="u")
            uT_sb = small.tile([128, 16, NUM_HEADS], f32, tag="uT")
            uT_bf = small.tile([128, 16, NUM_HEADS], bf16, tag="uTb")
            l_sb = small.tile([NUM_HEADS, 1], f32, tag="l")
            rl_sb = small.tile([NUM_HEADS, 1], f32, tag="rl")
            z_sb = small.tile([NUM_HEADS, SH], bf16, tag="z")
            zT_sb = small.tile([128, 2, NUM_HEADS], bf16, tag="zT")
            atp_sb = small.tile([128, NUM_HEADS], f32, tag="atp")
            atT_bf = small.tile([128, NUM_HEADS], bf16, tag="atTb")
            out_sb = small.tile([1, SH], f32, tag="out")

            # ---------------- dummy collective: absorb CC entry barrier ----
            nc.vector.memset(dum_sb[:], 0)
            nc.gpsimd.dma_start(bdum_in[:], dum_sb[:])
            nc.gpsimd.collective_compute(
                "AllGather", ALU.bypass, ins=[bdum_in[:].opt()],
                outs=[bdum_out[:].opt()], replica_groups=RG)

            # ---------------- DMA priority stream (sync queue) -------------
            nc.sync.dma_start(x_sb[:], xq_d[:].rearrange("(f p) -> p f", p=128))
            nc.sync.dma_start(ident_b[:], identb_d[0:16, 0:16])
            nc.sync.dma_start(ident_f[:], identf_d[0:16, 0:16])
            nc.sync.dma_start(xo_sb[:], xo_d[:].rearrange("(a n) -> a n", a=1))

            wq_sb = wts.tile([128, 16, SH], bf16, tag="wq")
            wqm_sb = wts.tile([128, 2, QK], bf16, tag="wqm")
            wkm_sb = wts.tile([128, 16, SH], bf16, tag="wkm")
            wk_sb = wts.tile([128, 2, VO], bf16, tag="wk")
            nc.sync.dma_start(wq_sb[:], wq_d[:, :].rearrange("(kc p) n -> p kc n", p=128))
            nc.sync.dma_start(wqm_sb[:], wqm_d[:, :].rearrange("(nc2 p) m -> p nc2 m", p=128))
            nc.sync.dma_start(wkm_sb[:], wkm_d[:, :].rearrange("(h p) j -> p h j", p=128))
            nc.sync.dma_start(wk_sb[:], wk_d[:, :].rearrange("(jc p) n -> p jc n", p=128))

            # keys + rope tables, interleaved so rope can start early
            ck_sb = tabs.tile([128, 8, S_LOC], bf16, tag="ck")
            sk_sb = tabs.tile([128, 8, S_LOC], bf16, tag="sk")
            kt = [kbuf.tile([128, S_LOC], bf16, tag="kt", name=f"kt{i}")
                  for i in range(16)]
            for t in range(8):
                nc.sync.dma_start(
                    ck_sb[:, t, :],
                    ck_d[t * 128:(t + 1) * 128, :])
                nc.sync.dma_start(
                    sk_sb[:, t, :],
                    sk_d[t * 128:(t + 1) * 128, :])
                nc.sync.dma_start(kt[t][:], keysT_d[t * 128:(t + 1) * 128, :])
                nc.sync.dma_start(
                    kt[t + 8][:], keysT_d[(t + 8) * 128:(t + 9) * 128, :])

            # states + tables (cs/ss reuse the ck/sk slots after keys rope)
            cs_sb = tabs.tile([128, 8, HALF], bf16, tag="ck")
            ss_sb = tabs.tile([128, 8, HALF], bf16, tag="sk")
            st = [sbuf_s.tile([128, VO], bf16, tag="st", name=f"st{i}")
                  for i in range(8)]
            for t in range(8):
                nc.sync.dma_start(cs_sb[:, t, :], cs_d[t * 128:(t + 1) * 128, :])
                nc.sync.dma_start(ss_sb[:, t, :], ss_d[t * 128:(t + 1) * 128, :])
                nc.sync.dma_start(st[t][:], states_d[t * 128:(t + 1) * 128, :])

            # epilogue weights (separate pool: no reuse wait)
            wvT_sb = wts2.tile([128, 16, SH], bf16, tag="wvT")
            wvm_sb = wts2.tile([128, 2, VO], bf16, tag="wvm")
            woT_sb = wts2.tile([128, 16, SH], bf16, tag="woT")
            nc.sync.dma_start(wvT_sb[:], wvT_d[:, :].rearrange("(ic p) n -> p ic n", p=128))
            nc.sync.dma_start(wvm_sb[:], wvm_d[:, :].rearrange("(jc p) m -> p jc m", p=128))
            nc.sync.dma_start(woT_sb[:], woT_d[:, :].rearrange("(mc p) n -> p mc n", p=128))

            # ---------------- qT = (x @ Wq[rs].T)^T  local shard [128, 2] ------
            for nc2 in range(2):
                qt_ps2 = psB.tile([128, 1], f32, tag="pB", name=f"qt_ps2_{nc2}")
                for kc in range(16):
                    nc.tensor.matmul(qt_ps2[:], wq_sb[:, kc, nc2 * 128:(nc2 + 1) * 128],
                                     x_sb[:, kc:kc + 1], start=(kc == 0), stop=(kc == 15))
                nc.scalar.activation(qT_sb[:, nc2:nc2 + 1], qt_ps2[:], AF.Copy)

            # ---------------- qh partial [m%128, 16] = Wq_mha[:,rs] q_shard ----
            qh_ps = psB.tile([128, NUM_HEADS], f32, tag="pB")
            for h in range(NUM_HEADS):
                for nc2 in range(2):
                    nc.tensor.matmul(qh_ps[:, h:h + 1],
                                     wqm_sb[:, nc2, h * 128:(h + 1) * 128],
                                     qT_sb[:, nc2:nc2 + 1],
                                     start=(nc2 == 0), stop=(nc2 == 1))
            nc.scalar.activation(qhp_sb[:], qh_ps[:], AF.Copy)
            nc.gpsimd.dma_start(bqh_in[:], qhp_sb[:])
            nc.gpsimd.collective_compute(
                "AllReduce", ALU.add, ins=[bqh_in[:].opt()],
                outs=[bqh_out[:].opt()], replica_groups=RG)
            nc.gpsimd.dma_start(qhT_sb[:], bqh_out[:, :])

            # ---------------- tmpT [j%128, jc, h] = Wk_mha[:,rs]^T qh ---------
            tmpT_ps = [psB.tile([128, NUM_HEADS], f32, tag="pB", name=f"tmpT_ps{j}")
                       for j in range(2)]
            for h in range(NUM_HEADS):
                for jc in range(2):
                    nc.tensor.matmul(tmpT_ps[jc][:, h:h + 1],
                                     wkm_sb[:, h, jc * 128:(jc + 1) * 128],
                                     qhT_sb[:, h:h + 1], start=True, stop=True)
            for jc in range(2):
                nc.scalar.activation(tmpT_sb[:, jc, :], tmpT_ps[jc][:], AF.Copy)

            # ---------------- qtil partial = Wk[rs]^T-contract --------------
            for ic in range(16):
                qt_ps = psB.tile([128, NUM_HEADS], f32, tag="pB")
                for jc in range(2):
                    nc.tensor.matmul(qt_ps[:], wk_sb[:, jc, ic * 128:(ic + 1) * 128],
                                     tmpT_sb[:, jc, :], start=(jc == 0), stop=(jc == 1))
                nc.scalar.activation(qtp_sb[:, ic, :], qt_ps[:], AF.Copy)
            nc.gpsimd.dma_start(
                bqt_in[:, :, :].rearrange("ic p h -> p ic h"), qtp_sb[:])
            nc.gpsimd.collective_compute(
                "AllReduce", ALU.add, ins=[bqt_in[:].opt()],
                outs=[bqt_out[:].opt()], replica_groups=RG)
            nc.gpsimd.dma_start(
                qtilT_sb[:], bqt_out[:, :, :].rearrange("ic p h -> p ic h"))

            # ---------------- rope keys in place (pairs t, t+8) -------------
            for t in range(8):
                a, b = kt[t], kt[t + 8]
                t1 = tmps.tile([128, S_LOC], bf16, tag="rt")
                t2 = tmps.tile([128, S_LOC], bf16, tag="rt")
                nc.vector.tensor_mul(t1[:], a[:], sk_sb[:, t, :])
                nc.vector.tensor_mul(a[:], a[:], ck_sb[:, t, :])
                nc.vector.tensor_mul(t2[:], b[:], sk_sb[:, t, :])
                nc.vector.tensor_sub(a[:], a[:], t2[:])
                nc.vector.tensor_mul(b[:], b[:], ck_sb[:, t, :])
                nc.vector.tensor_add(b[:], b[:], t1[:])

            # ---------------- logits + exp ----------------
            for sc in range(2):
                lg_ps = psA.tile([NUM_HEADS, 512], f32, tag="pA")
                for ic in range(16):
                    nc.tensor.matmul(lg_ps[:], qtilT_sb[:, ic, :],
                                     kt[ic][:, sc * 512:(sc + 1) * 512],
                                     start=(ic == 0), stop=(ic == 15))
                nc.scalar.activation(w_sb[:, sc * 512:(sc + 1) * 512], lg_ps[:],
                                     AF.Exp, scale=SCALE,
                                     accum_out=(l0_sb[:] if sc == 0 else l1_sb[:]))
            nc.vector.tensor_add(lp_sb[:], l0_sb[:], l1_sb[:])

            # wT via PE transpose: [16,128] slices -> [128,16]
            for sb in range(8):
                tr_ps = psB.tile([128, NUM_HEADS], bf16, tag="pB")
                nc.tensor.transpose(tr_ps[:], w_sb[:, sb * 128:(sb + 1) * 128],
                                    ident_b[:])
                nc.scalar.activation(wT_sb[:, sb, :], tr_ps[:], AF.Copy)

            # ---------------- rope states in place -------------
            for t in range(8):
                s_ = st[t]
                a = s_[:, 0:HALF]
                b = s_[:, HALF:VO]
                t1 = tmps.tile([128, HALF], bf16, tag="rt")
                t2 = tmps.tile([128, HALF], bf16, tag="rt")
                nc.vector.tensor_mul(t1[:], a, ss_sb[:, t, :])
                nc.vector.tensor_mul(a, a, cs_sb[:, t, :])
                nc.vector.tensor_mul(t2[:], b, ss_sb[:, t, :])
                nc.vector.tensor_sub(a, a, t2[:])
                nc.vector.tensor_mul(b, b, cs_sb[:, t, :])
                nc.vector.tensor_add(b, b, t1[:])

            # ---------------- u = wT.T @ states_pe  [16, 2048] f32 ----------
            u_ps = [psA.tile([NUM_HEADS, 512], f32, tag="pA", name=f"u_ps{i}")
                    for i in range(4)]
            for sb in range(8):
                for nch in range(4):
                    nc.tensor.matmul(u_ps[nch][:], wT_sb[:, sb, :],
                                     st[sb][:, nch * 512:(nch + 1) * 512],
                                     start=(sb == 0), stop=(sb == 7))
            for nch in range(4):
                nc.scalar.activation(u_sb[:, nch * 512:(nch + 1) * 512],
                                     u_ps[nch][:], AF.Copy)

            # uT via PE transpose (f32), then AR(uT|l)
            for ic in range(16):
                tr_ps = psB.tile([128, NUM_HEADS], f32, tag="pB")
                nc.tensor.transpose(tr_ps[:], u_sb[:, ic * 128:(ic + 1) * 128],
                                    ident_f[:])
                nc.scalar.activation(uT_sb[:, ic, :], tr_ps[:], AF.Copy)
            nc.gpsimd.dma_start(
                bu_in[:, 0:256].rearrange("p (ic h) -> p ic h", ic=16), uT_sb[:])
            nc.gpsimd.dma_start(bu_in[0:NUM_HEADS, 256:257], lp_sb[:])
            nc.gpsimd.collective_compute(
                "AllReduce", ALU.add, ins=[bu_in[:].opt()],
                outs=[bu_out[:].opt()], replica_groups=RG)
            nc.gpsimd.dma_start(
                uT_bf[:], bu_out[:, 0:256].rearrange("p (ic h) -> p ic h", ic=16))
            nc.gpsimd.dma_start(l_sb[:], bu_out[0:NUM_HEADS, 256:257])
            nc.vector.reciprocal(rl_sb[:], l_sb[:])

            # ---------------- z = (u @ Wv[rs].T) / l  [16, 256] -------------
            z_ps = psB.tile([NUM_HEADS, SH], f32, tag="pB")
            for ic in range(16):
                nc.tensor.matmul(z_ps[:], uT_bf[:, ic, :], wvT_sb[:, ic, :],
                                 start=(ic == 0), stop=(ic == 15))
            nc.scalar.activation(z_sb[:], z_ps[:], AF.Copy, scale=rl_sb[:])

            # zT
            for jc in range(2):
                tr_ps = psB.tile([128, NUM_HEADS], bf16, tag="pB")
                nc.tensor.transpose(tr_ps[:], z_sb[:, jc * 128:(jc + 1) * 128],
                                    ident_b[:])
                nc.scalar.activation(zT_sb[:, jc, :], tr_ps[:], AF.Copy)

            # ---------------- attn partial [128 d, 16 h] --------------------
            at_ps = psB.tile([128, NUM_HEADS], f32, tag="pB")
            for h in range(NUM_HEADS):
                for jc in range(2):
                    nc.tensor.matmul(at_ps[:, h:h + 1],
                                     wvm_sb[:, jc, h * 128:(h + 1) * 128],
                                     zT_sb[:, jc, h:h + 1],
                                     start=(jc == 0), stop=(jc == 1))
            nc.scalar.activation(atp_sb[:], at_ps[:], AF.Copy)
            nc.gpsimd.dma_start(bat_in[:], atp_sb[:])
            nc.gpsimd.collective_compute(
                "AllReduce", ALU.add, ins=[bat_in[:].opt()],
                outs=[bat_out[:].opt()], replica_groups=RG)
            nc.gpsimd.dma_start(atT_bf[:], bat_out[:, :])

            if DEBUG:
                nc.sync.dma_start(dqt_d[:, :, :], bqt_out[:, :, :])

            # ---------------- out = attn @ Wo[rs].T + x[rs] ------------------
            o_ps = psB.tile([1, SH], f32, tag="pB")
            for h in range(NUM_HEADS):
                nc.tensor.matmul(o_ps[:], atT_bf[:, h:h + 1], woT_sb[:, h, :],
                                 start=(h == 0), stop=(h == NUM_HEADS - 1))
            nc.vector.tensor_add(out_sb[:], o_ps[:], xo_sb[:])
            nc.sync.dma_start(out_d[:, :], out_sb[:])

    nc.compile()
    return nc


def _tables():
    half = HALF
    freqs = 1.0 / (ROPE_THETA ** (np.arange(half, dtype=np.float32) * 2.0 / VO))
    ang = np.outer(np.arange(S, dtype=np.float32), freqs).astype(np.float32)
    return np.cos(ang), np.sin(ang)


def kernel(x, keys, states, Wq, Wk, Wv, Wq_mha, Wk_mha, Wv_mha, Wo):
    from concourse import bass_utils

    if "nc" not in _cache:
        _cache["nc"] = _build()
    nc = _cache["nc"]

    x = np.asarray(x, np.float32)
    keys = np.asarray(keys, np.float32)
    states = np.asarray(states, np.float32)
    cos_t, sin_t = _tables()

    ident = np.eye(128, dtype=np.float32)
    in_maps = []
    for c in range(NC):
        rs = slice(c * SH, (c + 1) * SH)
        ss_ = slice(c * S_LOC, (c + 1) * S_LOC)
        cosc = cos_t[ss_]
        sinc = sin_t[ss_]
        m = {
            "xq": x.astype(BF16),
            "xo": np.ascontiguousarray(x[rs]),
            "identb": ident.astype(BF16),
            "identf": ident,
            "wq": np.ascontiguousarray(Wq[rs].T).astype(BF16),
            "wqm": np.ascontiguousarray(Wq_mha[:, rs].T).astype(BF16),
            "wkm": np.ascontiguousarray(Wk_mha[:, rs]).astype(BF16),
            "wk": np.ascontiguousarray(Wk[rs]).astype(BF16),
            "keysT": np.ascontiguousarray(keys[ss_].T).astype(BF16),
            "ck": np.ascontiguousarray(cosc.T).astype(BF16),
            "sk": np.ascontiguousarray(sinc.T).astype(BF16),
            "states": np.ascontiguousarray(states[ss_]).astype(BF16),
            "cs": np.ascontiguousarray(cosc).astype(BF16),
            "ss": np.ascontiguousarray(sinc).astype(BF16),
            "wvT": np.ascontiguousarray(Wv[rs].T).astype(BF16),
            "wvm": np.ascontiguousarray(Wv_mha[:, rs].T).astype(BF16),
            "woT": np.ascontiguousarray(Wo[rs].T).astype(BF16),
        }
        in_maps.append(m)

    global _last_in_maps, _last_res
    _last_in_maps = in_maps
    res = bass_utils.run_bass_kernel_spmd(nc, in_maps, core_ids=list(range(NC)))
    _last_res = res
    out = np.concatenate([np.asarray(res.results[c]["out"]).reshape(-1) for c in range(NC)])
    return out[None, :].astype(np.float32)
